# revision 8
# baseline (speedup 1.0000x reference)
"""Multi-head causal self-attention on 8 Trainium2 NeuronCores.

Problem: X[4,2048,1024], per-head Wq/Wk/Wv[16,1024,64], Wo[1024,1024], bo[1024].
    out = OutProj(concat_heads(softmax_causal(Q K^T / 8) V))

Sharding: 8 cores = 4 batches x 2 head-groups (8 heads each). Each core
computes its batch's attention for its 8 heads plus the partial output
projection over its 512 concat features; host sums the two partials per
batch and adds the bias.

Per-core kernel (all matmuls in float32r, transposed "feature-on-partition"
space so the softmax reduction lands on the free dimension):
  qT/kT per head-pair  [128, T]  = Wpair^T  x  X^T
  v    per s-tile      [128, 8*65] = X^T^T  x  Wv_all (65th col later set to 1)
  ST block [s=128, t=512] = kT_slice^T @ qT_slice   (row-packed pairs)
  expST = exp(ST/8) (ScalarE), causal-masked via gpsimd affine_select
  avT [65, 512] += [V|1]^T @ expST   -> rows 0:64 = (A@V)^T, row 64 = softmax sums
  normalize via 1/sums broadcast (rank-1 fp32 matmul) and write concatT
  partial = concatT^T @ WoST  (accumulated over 4 feature chunks)
"""

import os
import sys

for _p in ("/opt/trn_rl_repo", "/root/.axon_site/_ro/trn_rl_repo"):
    if os.path.isdir(_p) and _p not in sys.path:
        sys.path.append(_p)

import numpy as np

import concourse.bass as bass
import concourse.mybir as mybir
from concourse import bacc
import concourse.tile as tile

B, T, D, H, K = 4, 2048, 1024, 16, 64
HG = 8          # heads per core
NPAIR = 4       # head pairs per core
P = 128
DC = D // P     # 8 contraction chunks for the projections
NS = T // P     # 16 key tiles
NT = T // 512   # 4 query tiles of 512
F32 = mybir.dt.float32
F32R = mybir.dt.float32r


def build_module():
    nc = bacc.Bacc("TRN2")
    XT = nc.dram_tensor("xt", [D, T], F32R, kind="ExternalInput").ap()
    WQ = nc.dram_tensor("wq", [NPAIR, D, P], F32R, kind="ExternalInput").ap()
    WK = nc.dram_tensor("wk", [NPAIR, D, P], F32R, kind="ExternalInput").ap()
    WV = nc.dram_tensor("wv", [D, HG * K], F32R, kind="ExternalInput").ap()
    WO = nc.dram_tensor("wo", [HG * K, D], F32R, kind="ExternalInput").ap()
    OUT = nc.dram_tensor("out", [T, D], F32, kind="ExternalOutput").ap()

    with tile.TileContext(nc) as tc:
        with tc.tile_pool(name="persist", bufs=1) as pp:
            xt_sb = pp.tile([P, DC, T], F32R)           # X^T, 64 KB/partition
            v_sb = pp.tile([P, NS, HG * (K + 1)], F32R)  # V + ones col per head
            concat_sb = pp.tile([P, NPAIR, T], F32R)     # concat(heads)^T
            ones_sb = pp.tile([1, K], F32)

            xt_r = XT.rearrange("(c p) t -> c p t", p=P)
            for c in range(DC):
                nc.sync.dma_start(out=xt_sb[:, c, :], in_=xt_r[c])
            nc.vector.memset(ones_sb, 1.0)
            # ones column (index 64 of each head's 65-wide slot)
            v_slots = v_sb.rearrange("p s (h x) -> p s h x", x=K + 1)
            nc.vector.memset(v_slots[:, :, :, K : K + 1].bitcast(F32), 1.0)

            # ---- V projection, all 8 heads at once ----
            with (
                tc.tile_pool(name="wvp", bufs=1) as wvp,
                tc.tile_pool(name="psv", bufs=1, space="PSUM") as psv,
            ):
                wv_sb = wvp.tile([P, DC, HG * K], F32R)
                nc.sync.dma_start(
                    out=wv_sb, in_=WV.rearrange("(c p) n -> p c n", p=P)
                )
                for s in range(NS):
                    ps = psv.tile([P, HG * K], F32, tag="mm", bufs=3)
                    for c in range(DC):
                        nc.tensor.matmul(
                            ps,
                            xt_sb[:, c, s * P : (s + 1) * P],
                            wv_sb[:, c, :],
                            start=(c == 0),
                            stop=(c == DC - 1),
                        )
                    nc.vector.tensor_copy(
                        v_slots[:, s, :, 0:K],
                        ps.rearrange("p (h k) -> p h k", k=K),
                    )

            # ---- attention, one head pair at a time ----
            with (
                tc.tile_pool(name="attn", bufs=1) as ap_,
                tc.tile_pool(name="psa", bufs=1, space="PSUM") as psa,
            ):
                for pr in range(NPAIR):
                    wq_sb = ap_.tile([P, DC, P], F32R, tag="wq", bufs=2)
                    wk_sb = ap_.tile([P, DC, P], F32R, tag="wk", bufs=2)
                    nc.sync.dma_start(
                        out=wq_sb, in_=WQ[pr].rearrange("(c p) m -> p c m", p=P)
                    )
                    nc.sync.dma_start(
                        out=wk_sb, in_=WK[pr].rearrange("(c p) m -> p c m", p=P)
                    )
                    q_sb = ap_.tile([P, T], F32R, tag="q", bufs=2)
                    k_sb = ap_.tile([P, T], F32R, tag="k", bufs=2)
                    for w_sb, qk_sb in ((wq_sb, q_sb), (wk_sb, k_sb)):
                        for tt in range(NT):
                            ps = psa.tile([P, 512], F32, tag="mm", bufs=3)
                            for c in range(DC):
                                nc.tensor.matmul(
                                    ps,
                                    w_sb[:, c, :],
                                    xt_sb[:, c, tt * 512 : (tt + 1) * 512],
                                    start=(c == 0),
                                    stop=(c == DC - 1),
                                )
                            nc.vector.tensor_copy(
                                qk_sb[:, tt * 512 : (tt + 1) * 512], ps
                            )

                    for tt in range(NT):
                        avs = [
                            psa.tile(
                                [K + 1, 512], F32, tag="av", bufs=3,
                                name=f"av{pr}_{tt}_{h2}",
                            )
                            for h2 in range(2)
                        ]
                        n_s = 4 * tt + 4
                        for si in range(n_s):
                            for h in range(2):
                                lo, hi = h * K, (h + 1) * K
                                st = psa.tile([P, 512], F32, tag="mm", bufs=3)
                                nc.tensor.matmul(
                                    st,
                                    k_sb[lo:hi, si * P : (si + 1) * P],
                                    q_sb[lo:hi, tt * 512 : (tt + 1) * 512],
                                    start=True,
                                    stop=True,
                                )
                                ex = ap_.tile([P, 512], F32R, tag="exp", bufs=4)
                                nc.scalar.activation(
                                    ex, st, mybir.ActivationFunctionType.Exp,
                                    scale=0.125,
                                )
                                if si >= 4 * tt:  # block straddles the diagonal
                                    nc.gpsimd.affine_select(
                                        out=ex,
                                        in_=ex,
                                        compare_op=mybir.AluOpType.is_ge,
                                        fill=0.0,
                                        base=512 * tt - P * si,
                                        channel_multiplier=-1,
                                        pattern=[[1, 512]],
                                    )
                                slot = (2 * pr + h) * (K + 1)
                                nc.tensor.matmul(
                                    avs[h],
                                    v_sb[:, si, slot : slot + K + 1],
                                    ex,
                                    start=(si == 0),
                                    stop=(si == n_s - 1),
                                )
                        # normalize: rows 0:64 are (A@V)^T, row 64 is the sums
                        for h in range(2):
                            recip = ap_.tile([1, 512], F32, tag="recip", bufs=2)
                            nc.vector.reciprocal(recip, avs[h][K : K + 1, :])
                            bc = psa.tile([K, 512], F32, tag="bc", bufs=2)
                            # rank-1 fp32 matmul broadcasts recip to 64 rows
                            nc.tensor.matmul(
                                bc, ones_sb[0:1, :], recip, start=True, stop=True
                            )
                            bc_sb = ap_.tile([K, 512], F32, tag="bc_sb", bufs=2)
                            nc.vector.tensor_copy(bc_sb, bc)
                            cols = slice(tt * 512, (tt + 1) * 512)
                            if h == 0:
                                nc.vector.tensor_mul(
                                    concat_sb[0:K, pr, cols], avs[h][0:K, :], bc_sb
                                )
                            else:
                                # partition-shifted write via DMA bounce
                                tmp = ap_.tile([K, 512], F32R, tag="tmpb", bufs=2)
                                nc.vector.tensor_mul(tmp, avs[h][0:K, :], bc_sb)
                                nc.sync.dma_start(
                                    out=concat_sb[K:P, pr, cols], in_=tmp
                                )

            # ---- output projection (partial: this core's 512 features) ----
            with (
                tc.tile_pool(name="op", bufs=1) as op_,
                tc.tile_pool(name="pso", bufs=1, space="PSUM") as pso,
            ):
                wo_sb = op_.tile([P, NPAIR, D], F32R)
                nc.sync.dma_start(
                    out=wo_sb, in_=WO.rearrange("(s p) o -> p s o", p=P)
                )
                for t16 in range(T // P):
                    for oc in range(2):
                        ps = pso.tile([P, 512], F32, tag="mm", bufs=4)
                        for s4 in range(NPAIR):
                            nc.tensor.matmul(
                                ps,
                                concat_sb[:, s4, t16 * P : (t16 + 1) * P],
                                wo_sb[:, s4, oc * 512 : (oc + 1) * 512],
                                start=(s4 == 0),
                                stop=(s4 == NPAIR - 1),
                            )
                        st_o = op_.tile([P, 512], F32, tag="outst", bufs=3)
                        nc.vector.tensor_copy(st_o, ps)
                        nc.sync.dma_start(
                            out=OUT[
                                t16 * P : (t16 + 1) * P, oc * 512 : (oc + 1) * 512
                            ],
                            in_=st_o,
                        )
    nc.compile()
    return nc


def _to_f32r(a):
    """Round fp32 to the f32r grid (11-bit mantissa; low 12 bits zero)."""
    u = np.ascontiguousarray(a, dtype=np.float32).view(np.uint32)
    lsb = (u >> 12) & 1
    r = (u + 0x7FF + lsb) & 0xFFFFF000
    return r.view(np.float32)


def _to_f32r(a):
    """Round fp32 to the f32r grid (11-bit mantissa; low 12 bits zero)."""
    u = np.ascontiguousarray(a, dtype=np.float32).view(np.uint32)
    lsb = (u >> 12) & 1
    r = (u + 0x7FF + lsb) & 0xFFFFF000
    return r.view(np.float32)


def shard_inputs(X, Wq, Wk, Wv, Wo):
    """Host-side shard prep: core c handles batch c//2, head group c%2."""
    in_maps = []
    for c in range(8):
        b, g = c // 2, c % 2
        heads = range(g * HG, (g + 1) * HG)
        wq = np.stack(
            [
                np.concatenate([Wq[g * HG + 2 * p], Wq[g * HG + 2 * p + 1]], axis=1)
                for p in range(NPAIR)
            ]
        )
        wk = np.stack(
            [
                np.concatenate([Wk[g * HG + 2 * p], Wk[g * HG + 2 * p + 1]], axis=1)
                for p in range(NPAIR)
            ]
        )
        wv = np.concatenate([Wv[h] for h in heads], axis=1)
        wo = Wo[:, g * 512 : (g + 1) * 512].T
        in_maps.append(
            {
                "xt": _to_f32r(X[b].T),
                "wq": _to_f32r(wq),
                "wk": _to_f32r(wk),
                "wv": _to_f32r(wv),
                "wo": _to_f32r(wo),
            }
        )
    return in_maps


_MODULE = None


def _get_module():
    global _MODULE
    if _MODULE is None:
        _MODULE = build_module()
    return _MODULE


def kernel(X, Wq, Wk, Wv, Wo, bo, _want_results=None):
    from concourse.bass_utils import run_bass_kernel_spmd

    nc = _get_module()
    in_maps = shard_inputs(
        np.asarray(X), np.asarray(Wq), np.asarray(Wk), np.asarray(Wv), np.asarray(Wo)
    )
    res = run_bass_kernel_spmd(nc, in_maps, core_ids=list(range(8)))
    if _want_results is not None:
        _want_results.append(res)
    out = np.empty((B, T, H * K), dtype=np.float32)
    bo = np.asarray(bo, dtype=np.float32)
    for b in range(B):
        out[b] = res.results[2 * b]["out"] + res.results[2 * b + 1]["out"] + bo
    return out


# revision 9
# speedup vs baseline: 1.3712x; 1.3712x over previous
"""Multi-head causal self-attention on 8 Trainium2 NeuronCores.

Problem: X[4,2048,1024], per-head Wq/Wk/Wv[16,1024,64], Wo[1024,1024], bo[1024].
    out = OutProj(concat_heads(softmax_causal(Q K^T / 8) V))

Sharding: 8 cores = 4 batches x 2 head-groups (8 heads each). Each core
computes its batch's attention for its 8 heads plus the partial output
projection over its 512 concat features; host sums the two partials per
batch and adds the bias.

Per-core kernel (matmul operands in fp16 — 1 cycle/row on TensorE and
fp32 PSUM accumulation; softmax runs in the transposed
"feature-on-partition" space so its reduction lands on the free dim):
  qT/kT per head-pair  [128, T]  = Wpair^T  x  X^T
  v    per s-tile      [128, 8*65] = X^T^T  x  Wv_all (65th col set to 1)
  ST block [s=128, t=512] = kT_slice^T @ qT_slice   (row-packed head pairs)
  expST = exp(ST/8) (ScalarE), causal-masked via gpsimd affine_select
  avT [65, 512] += [V|1]^T @ expST   -> rows 0:64 = (A@V)^T, row 64 = sums
  normalize via 1/sums broadcast (rank-1 fp32 matmul) and write concatT
  partial = concatT^T @ WoST  (accumulated over 4 feature chunks)
"""

import os
import sys

for _p in ("/opt/trn_rl_repo", "/root/.axon_site/_ro/trn_rl_repo"):
    if os.path.isdir(_p) and _p not in sys.path:
        sys.path.append(_p)

import numpy as np

import concourse.mybir as mybir
import concourse.tile as tile
from concourse import bacc

B, T, D, H, K = 4, 2048, 1024, 16, 64
HG = 8          # heads per core
NPAIR = 4       # head pairs per core
P = 128
DC = D // P     # 8 contraction chunks for the projections
NS = T // P     # 16 key tiles
NT = T // 512   # 4 query tiles of 512
F32 = mybir.dt.float32
F16 = mybir.dt.float16


def build_module():
    nc = bacc.Bacc("TRN2")
    XT = nc.dram_tensor("xt", [D, T], F16, kind="ExternalInput").ap()
    WQ = nc.dram_tensor("wq", [NPAIR, D, P], F16, kind="ExternalInput").ap()
    WK = nc.dram_tensor("wk", [NPAIR, D, P], F16, kind="ExternalInput").ap()
    WV = nc.dram_tensor("wv", [D, HG * K], F16, kind="ExternalInput").ap()
    WO = nc.dram_tensor("wo", [HG * K, D], F16, kind="ExternalInput").ap()
    OUT = nc.dram_tensor("out", [T, D], F32, kind="ExternalOutput").ap()

    with tile.TileContext(nc) as tc:
        with tc.tile_pool(name="persist", bufs=1) as pp:
            xt_sb = pp.tile([P, DC, T], F16)            # X^T, 32 KB/partition
            v_sb = pp.tile([P, NS, HG * (K + 1)], F16)  # V + ones col per head
            concat_sb = pp.tile([P, NPAIR, T], F16)     # concat(heads)^T
            ones_sb = pp.tile([1, K], F32)

            xt_r = XT.rearrange("(c p) t -> c p t", p=P)
            for c in range(DC):
                nc.sync.dma_start(out=xt_sb[:, c, :], in_=xt_r[c])
            nc.vector.memset(ones_sb, 1.0)
            # ones column (index 64 of each head's 65-wide slot)
            v_slots = v_sb.rearrange("p s (h x) -> p s h x", x=K + 1)
            nc.vector.memset(v_slots[:, :, :, K : K + 1], 1.0)

            # ---- V projection, all 8 heads at once ----
            with (
                tc.tile_pool(name="wvp", bufs=1) as wvp,
                tc.tile_pool(name="psv", bufs=1, space="PSUM") as psv,
            ):
                wv_sb = wvp.tile([P, DC, HG * K], F16)
                nc.sync.dma_start(
                    out=wv_sb, in_=WV.rearrange("(c p) n -> p c n", p=P)
                )
                for s in range(NS):
                    ps = psv.tile([P, HG * K], F32, tag="mm", bufs=3)
                    for c in range(DC):
                        nc.tensor.matmul(
                            ps,
                            xt_sb[:, c, s * P : (s + 1) * P],
                            wv_sb[:, c, :],
                            start=(c == 0),
                            stop=(c == DC - 1),
                        )
                    nc.vector.tensor_copy(
                        v_slots[:, s, :, 0:K],
                        ps.rearrange("p (h k) -> p h k", k=K),
                    )

            # ---- attention, one head pair at a time ----
            with (
                tc.tile_pool(name="attn", bufs=1) as ap_,
                tc.tile_pool(name="psa", bufs=1, space="PSUM") as psa,
            ):
                for pr in range(NPAIR):
                    wq_sb = ap_.tile([P, DC, P], F16, tag="wq", bufs=2)
                    wk_sb = ap_.tile([P, DC, P], F16, tag="wk", bufs=2)
                    nc.sync.dma_start(
                        out=wq_sb, in_=WQ[pr].rearrange("(c p) m -> p c m", p=P)
                    )
                    nc.sync.dma_start(
                        out=wk_sb, in_=WK[pr].rearrange("(c p) m -> p c m", p=P)
                    )
                    q_sb = ap_.tile([P, T], F16, tag="q", bufs=2)
                    k_sb = ap_.tile([P, T], F16, tag="k", bufs=2)
                    for w_sb, qk_sb in ((wq_sb, q_sb), (wk_sb, k_sb)):
                        for tt in range(NT):
                            ps = psa.tile([P, 512], F32, tag="mm", bufs=3)
                            for c in range(DC):
                                nc.tensor.matmul(
                                    ps,
                                    w_sb[:, c, :],
                                    xt_sb[:, c, tt * 512 : (tt + 1) * 512],
                                    start=(c == 0),
                                    stop=(c == DC - 1),
                                )
                            nc.vector.tensor_copy(
                                qk_sb[:, tt * 512 : (tt + 1) * 512], ps
                            )

                    for tt in range(NT):
                        avs = [
                            psa.tile(
                                [K + 1, 512], F32, tag="av", bufs=3,
                                name=f"av{pr}_{tt}_{h2}",
                            )
                            for h2 in range(2)
                        ]
                        n_s = 4 * tt + 4
                        for si in range(n_s):
                            for h in range(2):
                                lo, hi = h * K, (h + 1) * K
                                st = psa.tile([P, 512], F32, tag="mm", bufs=3)
                                nc.tensor.matmul(
                                    st,
                                    k_sb[lo:hi, si * P : (si + 1) * P],
                                    q_sb[lo:hi, tt * 512 : (tt + 1) * 512],
                                    start=True,
                                    stop=True,
                                )
                                ex = ap_.tile([P, 512], F16, tag="exp", bufs=4)
                                nc.scalar.activation(
                                    ex, st, mybir.ActivationFunctionType.Exp,
                                    scale=0.125,
                                )
                                if si >= 4 * tt:  # block straddles the diagonal
                                    nc.gpsimd.affine_select(
                                        out=ex,
                                        in_=ex,
                                        compare_op=mybir.AluOpType.is_ge,
                                        fill=0.0,
                                        base=512 * tt - P * si,
                                        channel_multiplier=-1,
                                        pattern=[[1, 512]],
                                    )
                                slot = (2 * pr + h) * (K + 1)
                                nc.tensor.matmul(
                                    avs[h],
                                    v_sb[:, si, slot : slot + K + 1],
                                    ex,
                                    start=(si == 0),
                                    stop=(si == n_s - 1),
                                )
                        # normalize: rows 0:64 are (A@V)^T, row 64 is the sums
                        for h in range(2):
                            recip = ap_.tile([1, 512], F32, tag="recip", bufs=2)
                            nc.vector.reciprocal_approx_fast(
                                recip, avs[h][K : K + 1, :]
                            )
                            bc = psa.tile([K, 512], F32, tag="bc", bufs=2)
                            # rank-1 fp32 matmul broadcasts recip to 64 rows
                            nc.tensor.matmul(
                                bc, ones_sb[0:1, :], recip, start=True, stop=True
                            )
                            bc_sb = ap_.tile([K, 512], F32, tag="bc_sb", bufs=2)
                            nc.vector.tensor_copy(bc_sb, bc)
                            cols = slice(tt * 512, (tt + 1) * 512)
                            if h == 0:
                                nc.vector.tensor_mul(
                                    concat_sb[0:K, pr, cols], avs[h][0:K, :], bc_sb
                                )
                            else:
                                # partition-shifted write via DMA bounce
                                tmp = ap_.tile([K, 512], F16, tag="tmpb", bufs=2)
                                nc.vector.tensor_mul(tmp, avs[h][0:K, :], bc_sb)
                                nc.sync.dma_start(
                                    out=concat_sb[K:P, pr, cols], in_=tmp
                                )

            # ---- output projection (partial: this core's 512 features) ----
            with (
                tc.tile_pool(name="op", bufs=1) as op_,
                tc.tile_pool(name="pso", bufs=1, space="PSUM") as pso,
            ):
                wo_sb = op_.tile([P, NPAIR, D], F16)
                nc.sync.dma_start(
                    out=wo_sb, in_=WO.rearrange("(s p) o -> p s o", p=P)
                )
                for t16 in range(T // P):
                    for oc in range(2):
                        ps = pso.tile([P, 512], F32, tag="mm", bufs=4)
                        for s4 in range(NPAIR):
                            nc.tensor.matmul(
                                ps,
                                concat_sb[:, s4, t16 * P : (t16 + 1) * P],
                                wo_sb[:, s4, oc * 512 : (oc + 1) * 512],
                                start=(s4 == 0),
                                stop=(s4 == NPAIR - 1),
                            )
                        st_o = op_.tile([P, 512], F32, tag="outst", bufs=3)
                        nc.vector.tensor_copy(st_o, ps)
                        nc.sync.dma_start(
                            out=OUT[
                                t16 * P : (t16 + 1) * P, oc * 512 : (oc + 1) * 512
                            ],
                            in_=st_o,
                        )
    nc.compile()
    return nc


def shard_inputs(X, Wq, Wk, Wv, Wo):
    """Host-side shard prep: core c handles batch c//2, head group c%2."""
    in_maps = []
    for c in range(8):
        b, g = c // 2, c % 2
        heads = range(g * HG, (g + 1) * HG)
        wq = np.stack(
            [
                np.concatenate([Wq[g * HG + 2 * p], Wq[g * HG + 2 * p + 1]], axis=1)
                for p in range(NPAIR)
            ]
        )
        wk = np.stack(
            [
                np.concatenate([Wk[g * HG + 2 * p], Wk[g * HG + 2 * p + 1]], axis=1)
                for p in range(NPAIR)
            ]
        )
        wv = np.concatenate([Wv[h] for h in heads], axis=1)
        wo = Wo[:, g * 512 : (g + 1) * 512].T
        in_maps.append(
            {
                "xt": np.ascontiguousarray(X[b].T).astype(np.float16),
                "wq": np.ascontiguousarray(wq).astype(np.float16),
                "wk": np.ascontiguousarray(wk).astype(np.float16),
                "wv": np.ascontiguousarray(wv).astype(np.float16),
                "wo": np.ascontiguousarray(wo).astype(np.float16),
            }
        )
    return in_maps


_MODULE = None


def _get_module():
    global _MODULE
    if _MODULE is None:
        _MODULE = build_module()
    return _MODULE


def kernel(X, Wq, Wk, Wv, Wo, bo, _want_results=None):
    from concourse.bass_utils import run_bass_kernel_spmd

    nc = _get_module()
    in_maps = shard_inputs(
        np.asarray(X), np.asarray(Wq), np.asarray(Wk), np.asarray(Wv), np.asarray(Wo)
    )
    res = run_bass_kernel_spmd(nc, in_maps, core_ids=list(range(8)))
    if _want_results is not None:
        _want_results.append(res)
    out = np.empty((B, T, H * K), dtype=np.float32)
    bo = np.asarray(bo, dtype=np.float32)
    for b in range(B):
        out[b] = res.results[2 * b]["out"] + res.results[2 * b + 1]["out"] + bo
    return out
